# revision 1
# baseline (speedup 1.0000x reference)
"""Trainium2 Bass kernel for the AttentionBlock problem.

Full inputs -> full output. Internally sharded across 8 NeuronCores:
core c computes output rows [1024*c, 1024*(c+1)) (sequence-parallel over
queries); every core receives the full x (2 MB) so no on-device
collectives are needed.

Per-core algorithm (N=8192 keys, Nq=1024 queries, d=64):
  Qs^T = (R/8)^T x_q^T, K^T = E^T x^T            (fp32 PE matmuls)
  per 512-query chunk qc:
    pass 1: m_q = max_k (Qs K^T)[q, k]            (bf16 PE + DVE reduce)
    pass 2 (augmented, transposed):
      lhsT = [K^T; 1] (65 x 128 blocks), rhs = [Qs^T; -m]  (65 x 512)
      S^T_shifted = matmul -> PSUM (fp32), exp on ACT -> P^T (fp16)
      out_aug^T += matmul(lhsT=x_aug_j [128 x 65] fp16, rhs=P^T_j)
    out = out_aug^T[0:64] / out_aug^T[64] (PE transpose + reciprocal + mul)

The ones column of x_aug makes row 64 of out_aug^T the softmax
denominator; the -m row of the augmented Qs^T applies the max shift
inside the matmul (softmax is shift-invariant, so bf16 max error only
moves the shift, never the result).
"""

import numpy as np
from contextlib import ExitStack

import concourse.bass as bass
import concourse.tile as tile
from concourse import bacc, mybir

N = 8192
D = 64
DA = D + 1
NCORES = 8
NQ = N // NCORES          # 1024 queries per core
NKB = N // 128            # 64 key blocks
NSC = N // 512            # 16 key chunks of 512
QC = 512                  # query chunk (pass-2 moving dim)
NQC = NQ // QC            # 2

ST_DT = "f16x2p"          # score matmul: f32 | f32r | f16x3 | f16x2p (packed)
PV_DT = "f16"             # PV matmul dtype: f16 | f32r

F32 = mybir.dt.float32
F32R = mybir.dt.float32r
F16 = mybir.dt.float16
BF16 = mybir.dt.bfloat16


def build(st_dt=None, pv_dt=None):
    st_name = st_dt or ST_DT
    st_split = st_name in ("f16x3", "f16x2p")
    st_pack = st_name == "f16x2p"
    st_dt = {"f32": F32, "f32r": F32R, "f16x3": F16, "f16x2p": F16}[st_name]
    pv_dt = {"f16": F16, "f32r": F32R}[pv_dt or PV_DT]

    nc = bacc.Bacc("TRN2", target_bir_lowering=False, debug=False, num_devices=1)

    x_ap = nc.dram_tensor("x", [N, D], F32, kind="ExternalInput").ap()
    xT_ap = nc.dram_tensor("xT", [D, N], F32, kind="ExternalInput").ap()
    xqT_ap = nc.dram_tensor("xqT", [D, NQ], F32, kind="ExternalInput").ap()
    rp_ap = nc.dram_tensor("Rp", [D, D], F32, kind="ExternalInput").ap()
    e_ap = nc.dram_tensor("E", [D, D], F32, kind="ExternalInput").ap()
    id_ap = nc.dram_tensor("ident", [128, 128], F32, kind="ExternalInput").ap()
    ones_ap = nc.dram_tensor("ones", [1, N], F32, kind="ExternalInput").ap()
    ones16_ap = nc.dram_tensor("ones16", [1, N], F16, kind="ExternalInput").ap()
    DP = 72                   # x_aug block stride, 16-byte aligned in fp16
    xaug_ap = nc.dram_tensor("xaug", [N, DP], F16, kind="ExternalInput").ap()
    out_ap = nc.dram_tensor("out", [NQ, D], F32, kind="ExternalOutput").ap()

    with tile.TileContext(nc) as tc, ExitStack() as ctx:
        const = ctx.enter_context(tc.tile_pool(name="const", bufs=1))
        big = ctx.enter_context(tc.tile_pool(name="big", bufs=1))
        work = ctx.enter_context(tc.tile_pool(name="work", bufs=3))
        # PSUM budget (8 banks): ps1 [128,1024] x2 = 4, mm512 [128,512] x2 = 2,
        # po [65,512] x2 = 2.
        pp1 = ctx.enter_context(tc.tile_pool(name="pp1", bufs=2, space="PSUM"))
        pp = ctx.enter_context(tc.tile_pool(name="pp", bufs=2, space="PSUM"))
        pacc = ctx.enter_context(tc.tile_pool(name="pacc", bufs=2, space="PSUM"))

        # ---------------- input loads ----------------
        # small projection weights first so the first matmuls start early
        rp_sb = const.tile([D, D], F32)
        nc.sync.dma_start(rp_sb[:], rp_ap[:])
        e_sb = const.tile([D, D], F32)
        nc.sync.dma_start(e_sb[:], e_ap[:])
        xqt_sb = big.tile([D, NQ], F32)
        nc.sync.dma_start(xqt_sb[:], xqT_ap[:])
        xt_sb = big.tile([D, N], F32)
        for s in range(8):
            w = N // 8
            nc.sync.dma_start(xt_sb[:, s * w:(s + 1) * w], xT_ap[:, s * w:(s + 1) * w])
        ident = const.tile([128, 128], F32)
        nc.sync.dma_start(ident[:], id_ap[:])

        # x with ones column for the PV matmul, layout [128, (block, d_pad)]
        xaug_r = big.tile([128, NKB * DP], pv_dt)
        if pv_dt == F16:
            nc.sync.dma_start(
                xaug_r[:].rearrange("p (t d) -> p t d", d=DP),
                xaug_ap.rearrange("(t p) d -> p t d", p=128))
        else:
            xaug_f = big.tile([128, NKB * DA], F32)
            xaug_view_f = xaug_f[:].rearrange("p (t d) -> p t d", d=DA)
            nc.vector.memset(xaug_view_f[:, :, D:DA], 1.0)
            nc.sync.dma_start(xaug_view_f[:, :, 0:D],
                              x_ap.rearrange("(t p) d -> p t d", p=128))
            nc.vector.tensor_copy(xaug_r[:], xaug_f[:])
        xaug_v = xaug_r[:].rearrange("p (t d) -> p t d", d=DP)[:, :, 0:DA]

        # ---------------- projections ----------------
        # Qs^T first (2 chunks) so pass-1 lhsT is ready early.
        # K^T is split into 4 quarter tiles so pass-1 score matmuls (and
        # their DVE reductions) start after the first quarter instead of
        # waiting for the whole projection (Tile deps are tile-granular).
        NKQ = 4
        KW = N // NKQ
        qst_s = big.tile([DA, NQ], st_dt)
        qst_l = (big.tile([DA if st_pack else D, NQ], F16, name="qst_l")
                 if st_split else None)
        qst_hh = big.tile([128, NQ], F16, name="qst_hh") if st_pack else None
        qst_bf = qst_s if st_split else big.tile([D, NQ], BF16)
        for s in range(NQ // 512):
            pq_full = pp.tile([128, 512], F32, tag="mm512", name="pq")
            pq = pq_full[0:D, :]
            nc.tensor.matmul(pq[:], rp_sb[:], xqt_sb[:, s * 512:(s + 1) * 512],
                             start=True, stop=True)
            sl = slice(s * 512, (s + 1) * 512)
            if st_split:
                nc.scalar.copy(qst_s[0:D, sl], pq[:])
                nc.vector.tensor_tensor(
                    out=qst_l[0:D, sl], in0=pq[:], in1=qst_s[0:D, sl],
                    op=mybir.AluOpType.subtract)
                if st_pack:
                    nc.scalar.copy(qst_hh[0:D, sl], pq[:])
                    nc.scalar.copy(qst_hh[D:2 * D, sl], pq[:])
            else:
                nc.vector.tensor_copy(qst_s[0:D, sl], pq[:])
                nc.scalar.copy(qst_bf[:, sl], pq[:])

        kt_ss = [big.tile([DA, KW], st_dt, name=f"kt_s{q}") for q in range(NKQ)]
        kt_ls = ([big.tile([D, KW], F16, name=f"kt_l{q}") for q in range(NKQ)]
                 if st_split and not st_pack else None)
        kt_hl = ([big.tile([128, KW], F16, name=f"kt_hl{q}") for q in range(NKQ)]
                 if st_pack else None)
        kt_bfs = (kt_ss if st_split
                  else [big.tile([D, KW], BF16, name=f"kt_b{q}") for q in range(NKQ)])
        for q in range(NKQ):
            qw = slice(q * KW, (q + 1) * KW)
            if st_dt == F32:
                nc.sync.dma_start(kt_ss[q][D:DA, :], ones_ap[:, qw])
            elif st_split:
                nc.sync.dma_start(kt_ss[q][D:DA, :].bitcast(F32),
                                  ones16_ap[:, qw].bitcast(F32))
            else:
                ones_f = const.tile([1, KW], F32, tag="ones_f", name="ones_f")
                nc.vector.memset(ones_f[:], 1.0)
                nc.vector.tensor_copy(kt_ss[q][D:DA, :], ones_f[:])
        for s in range(NSC):
            q, so = divmod(s, NSC // NKQ)
            pk_full = pp.tile([128, 512], F32, tag="mm512", name="pk")
            pk = pk_full[0:D, :]
            nc.tensor.matmul(pk[:], e_sb[:], xt_sb[:, s * 512:(s + 1) * 512],
                             start=True, stop=True)
            sl = slice(so * 512, (so + 1) * 512)
            if st_split:
                # hi part on ACT, residual on DVE; the hi part doubles as the
                # pass-1 score operand (fp16 hi is more accurate than bf16)
                nc.scalar.copy(kt_ss[q][0:D, sl], pk[:])
                if st_pack:
                    nc.scalar.copy(kt_hl[q][0:D, sl], pk[:])
                    nc.vector.tensor_tensor(
                        out=kt_hl[q][D:2 * D, sl], in0=pk[:],
                        in1=kt_ss[q][0:D, sl], op=mybir.AluOpType.subtract)
                else:
                    nc.vector.tensor_tensor(
                        out=kt_ls[q][:, sl], in0=pk[:], in1=kt_ss[q][0:D, sl],
                        op=mybir.AluOpType.subtract)
            else:
                nc.vector.tensor_copy(kt_ss[q][0:D, sl], pk[:])
                nc.scalar.copy(kt_bfs[q][:, sl], pk[:])

        # -------- pass 1 for chunk 0, then pass 2 per chunk with the next
        # chunk's pass 1 interleaved into the j-loop. Engines execute a fixed
        # per-engine order, so emission order must keep chunk qc+1's max
        # reductions (DVE) flowing underneath chunk qc's pass 2 (PE/ACT)
        # without ever stalling the PE order on a ps1 slot.
        NRT = QC // 128                   # row-tiles per chunk (4)
        NG = NSC // 2                     # reduce groups per row-tile (8)
        mx_tiles = {}
        mxp_tiles = {}

        def emit_pass1_group(qc, gi):
            rt, g = divmod(gi, NG)
            if g == 0:
                mxp_tiles[qc] = work.tile([128, NG], F32, tag="mxp", name="mxp")
            mxp = mxp_tiles[qc]
            q0 = qc * QC + rt * 128
            ps1 = pp1.tile([128, 1024], F32, tag="ps1", name="ps1")
            for h in range(2):
                s = g * 2 + h
                kq, so = divmod(s, NSC // NKQ)
                nc.tensor.matmul(ps1[:, h * 512:(h + 1) * 512],
                                 qst_bf[0:D, q0:q0 + 128],
                                 kt_bfs[kq][0:D, so * 512:(so + 1) * 512],
                                 start=True, stop=True)
            nc.vector.reduce_max(mxp[:, g:g + 1], ps1[:],
                                 axis=mybir.AxisListType.X)
            if g == NG - 1:
                if qc not in mx_tiles:
                    mx_tiles[qc] = work.tile([128, NRT + 32], F32,
                                             tag="mx_all", name="mx_all")
                    nc.vector.memset(mx_tiles[qc][:], 0.0)
                nc.vector.reduce_max(mx_tiles[qc][:, rt:rt + 1], mxp[:],
                                     axis=mybir.AxisListType.X, negate=True)

        def emit_max_writeback(qc):
            # PSUM/SBUF reads must start at an aligned partition, so bring
            # each row-tile's -max to partition 0 with its own 32-wide
            # (non-degenerate) PE transpose of the zero-padded max tile,
            # then copy row 0 into qst_s row 64.
            for rt in range(NRT):
                pm_full = pp.tile([128, 512], F32, tag="mm512", name="pm")
                ps_m = pm_full[0:32, 0:128]
                nc.tensor.transpose(ps_m[:], mx_tiles[qc][:, rt:rt + 32],
                                    ident[:])
                sl = slice(qc * QC + rt * 128, qc * QC + (rt + 1) * 128)
                nc.vector.tensor_copy(
                    (qst_l if st_pack else qst_s)[D:DA, sl], ps_m[0:1, :])

        for gi in range(NRT * NG):
            emit_pass1_group(0, gi)
        emit_max_writeback(0)

        for qc in range(NQC):
            # pass 2, software-pipelined at emission so the PE order is
            # S_0, S_1, PV_0, S_2, PV_1, ... (PE never waits on an exp)
            po = pacc.tile([DA, QC], F32, tag="po")

            def emit_st(j):
                ps = pp.tile([128, QC], F32, tag="mm512", name="ps_st")
                kq, jo = divmod(j, NKB // NKQ)
                blk = slice(jo * 128, (jo + 1) * 128)
                qsl = slice(qc * QC, (qc + 1) * QC)
                if st_pack:
                    nc.tensor.matmul(ps[:], kt_hl[kq][:, blk], qst_hh[:, qsl],
                                     start=True, stop=False)
                    nc.tensor.matmul(ps[:], kt_ss[kq][:, blk], qst_l[:, qsl],
                                     start=False, stop=True)
                elif st_split:
                    nc.tensor.matmul(ps[:], kt_ss[kq][:, blk], qst_s[:, qsl],
                                     start=True, stop=False)
                    nc.tensor.matmul(ps[:], kt_ls[kq][:, blk], qst_s[0:D, qsl],
                                     start=False, stop=False)
                    nc.tensor.matmul(ps[:], kt_ss[kq][0:D, blk], qst_l[:, qsl],
                                     start=False, stop=True)
                else:
                    nc.tensor.matmul(ps[:], kt_ss[kq][:, blk], qst_s[:, qsl],
                                     start=True, stop=True)
                return ps

            ps_cur = emit_st(0)
            for j in range(NKB):
                pt = work.tile([128, QC], pv_dt, tag="pt")
                nc.scalar.activation(pt[:], ps_cur[:],
                                     mybir.ActivationFunctionType.Exp)
                if j + 1 < NKB:
                    ps_cur = emit_st(j + 1)
                nc.tensor.matmul(po[:], xaug_v[:, j, :], pt[:],
                                 start=(j == 0), stop=(j == NKB - 1))
                iv = NKB // (NRT * NG)   # j-iters per pass-1 group
                if qc + 1 < NQC and j % iv == iv - 1:
                    emit_pass1_group(qc + 1, j // iv)
            if qc + 1 < NQC:
                emit_max_writeback(qc + 1)

            # normalize: out[q, :] = po[0:64, q] / po[64, q]
            ot = work.tile([DA, QC], F32, tag="ot")
            nc.vector.tensor_copy(ot[:], po[:])
            for h in range(QC // 128):
                ptr_full = pp.tile([128, 512], F32, tag="mm512", name="ptr")
                ps_t = ptr_full[:, 0:DA]
                nc.tensor.transpose(ps_t[:], ot[:, h * 128:(h + 1) * 128],
                                    ident[0:DA, 0:DA])
                recip = work.tile([128, 1], F32, tag="recip")
                nc.vector.reciprocal(recip[:], ps_t[:, D:DA])
                o_sb = work.tile([128, D], F32, tag="o_sb")
                nc.vector.tensor_scalar_mul(o_sb[:], ps_t[:, 0:D], recip[:])
                r0 = qc * QC + h * 128
                nc.sync.dma_start(out_ap[r0:r0 + 128, :], o_sb[:])

    nc.compile()
    return nc


_CACHE = {}


def _get_nc():
    key = (ST_DT, PV_DT)
    if key not in _CACHE:
        _CACHE[key] = build(*key)
    return _CACHE[key]


def kernel(x, rotation_params, entangle_params, _trace=False, _nc=None):
    from concourse.bass_utils import run_bass_kernel_spmd

    x = np.ascontiguousarray(x, dtype=np.float32)
    rp = np.ascontiguousarray(rotation_params, dtype=np.float32) / 8.0
    e = np.ascontiguousarray(entangle_params, dtype=np.float32)
    xT = np.ascontiguousarray(x.T)

    nc = _nc if _nc is not None else _get_nc()
    ones = np.ones((1, N), dtype=np.float32)
    xaug16 = np.zeros((N, 72), dtype=np.float16)
    xaug16[:, :D] = x.astype(np.float16)
    xaug16[:, D] = 1.0

    in_maps = []
    for c in range(NCORES):
        in_maps.append({
            "x": x,
            "xT": xT,
            "xqT": np.ascontiguousarray(xT[:, c * NQ:(c + 1) * NQ]),
            "Rp": rp,
            "E": e,
            "ident": np.eye(128, dtype=np.float32),
            "ones": ones,
            "ones16": ones.astype(np.float16),
            "xaug": xaug16,
        })
    res = run_bass_kernel_spmd(nc, in_maps, core_ids=list(range(NCORES)),
                               trace=_trace)
    out = np.concatenate([res.results[c]["out"] for c in range(NCORES)], axis=0)
    if _trace:
        return out, res
    return out



# revision 5
# speedup vs baseline: 1.1905x; 1.1905x over previous
"""Trainium2 Bass kernel for the AttentionBlock problem (v2, all-bf16 PE).

Full inputs -> full output. Internally sharded across 8 NeuronCores:
core c computes output rows [1024*c, 1024*(c+1)) (sequence-parallel over
queries); every core receives the full key-side x so no on-device
collectives are needed.

Math (per core, N=8192 keys, Nq=1024 queries, d=64), everything on the
PE in full-rate bf16 (fp16 matmuls run at HALF rate on trn2, fp32 at
~1/5 - the v1 kernel was built on fp16/fp32 and measured 290us):

  projections (bf16 hi/lo split, x pre-split on host):
    Q = x@R/8 = [Rh;Rl]^T [xh;xh] + Rh^T xl   (2 MMs per 512 cols)
  scores, 2 bf16 MMs per 128-key block:
    MM1 = [K_hi; 1]^T [Q_hi; -m]      (65 contraction rows, has max shift)
    MM2 = [K_hi; K_lo]^T [Q_lo; Q_hi] (128 rows: hi*lo + lo*hi cross terms)
  P = bf16(exp(S))  - bf16's e^88 range absorbs max-estimate slop
  PV: out_aug^T += xaug_j^T P_j  (xaug = [x_bf16, 1], 1 bf16 MM per block)
  out = out_aug[0:64] / out_aug[64]

  max pass (pass-1): bf16 K_hi.Q_hi scores; chunk 0 (the serial prefix)
  splits the per-query max between DVE (reduce_max, row-tiles 0,1) and
  ACT (log-sum-exp with T=10: m = T*ln sum exp(l/T) in [max, max+28],
  row-tiles 2,3) so both engines chew the prefix in parallel; chunk 1's
  pass-1 runs all-DVE interleaved under chunk 0's main loop.

Measured numeric margin (CPU sim of this exact quantization): rel err
3.1e-3 vs the 2e-2 gate.
"""

import numpy as np
from contextlib import ExitStack

import concourse.bass as bass
import concourse.tile as tile
from concourse import bacc, mybir

N = 8192
D = 64
NCORES = 8
NQ = N // NCORES          # 1024 queries per core
NKB = N // 128            # 64 key blocks
QC = 512                  # query chunk
NQC = NQ // QC            # 2
DP = 72                   # xaug row stride (16B aligned in bf16)
T_LSE = 10.0              # lse temperature for the ACT max-proxy

F32 = mybir.dt.float32
BF16 = mybir.dt.bfloat16
AX = mybir.AxisListType.X
SUB = mybir.AluOpType.subtract
EXP = mybir.ActivationFunctionType.Exp
LN = mybir.ActivationFunctionType.Ln


def build():
    nc = bacc.Bacc("TRN2", target_bir_lowering=False, debug=False, num_devices=1)

    ones_ap = nc.dram_tensor("ones16", [1, N], BF16, kind="ExternalInput").ap()
    rhl_ap = nc.dram_tensor("rhl", [128, D], BF16, kind="ExternalInput").ap()
    rh_ap = nc.dram_tensor("rh", [D, D], BF16, kind="ExternalInput").ap()
    ehl_ap = nc.dram_tensor("ehl", [128, D], BF16, kind="ExternalInput").ap()
    eh_ap = nc.dram_tensor("eh", [D, D], BF16, kind="ExternalInput").ap()
    xqhh_ap = nc.dram_tensor("xqhh", [128, NQ], BF16, kind="ExternalInput").ap()
    xqlo_ap = nc.dram_tensor("xqlo", [D, NQ], BF16, kind="ExternalInput").ap()
    xthh_ap = nc.dram_tensor("xthh", [128, N], BF16, kind="ExternalInput").ap()
    xtlo_ap = nc.dram_tensor("xtlo", [D, N], BF16, kind="ExternalInput").ap()
    id_ap = nc.dram_tensor("ident", [128, 128], F32, kind="ExternalInput").ap()
    xaug_ap = nc.dram_tensor("xaug", [128, NKB * DP], BF16,
                             kind="ExternalInput").ap()
    out_ap = nc.dram_tensor("out", [NQ, D], F32, kind="ExternalOutput").ap()
    dbg_ap = nc.dram_tensor("dbg_m", [1, NQ], BF16, kind="ExternalOutput").ap()

    with tile.TileContext(nc) as tc, ExitStack() as ctx:
        const = ctx.enter_context(tc.tile_pool(name="const", bufs=1))
        big = ctx.enter_context(tc.tile_pool(name="big", bufs=1))
        work = ctx.enter_context(tc.tile_pool(name="work", bufs=3))
        small = ctx.enter_context(tc.tile_pool(name="small", bufs=2))
        # PSUM: ppool 3x[128,1024] = 6 banks, pacc 2x[65,512] = 2 banks
        ppool = ctx.enter_context(tc.tile_pool(name="pp", bufs=3, space="PSUM"))
        pacc = ctx.enter_context(tc.tile_pool(name="pacc", bufs=2, space="PSUM"))

        # ---------------- tiles + input DMA ----------------
        # kt_s quarters [65, 2048]: K_hi rows 0..63, ones row 64.
        # kt_l quarters [128, 2048]: K_hi rows 0..63, K_lo rows 64..127.
        # (quartered so pass-1/pass-2 start before the whole projection is
        # done - tile deps are tile-granular)
        kt_s = [big.tile([D + 1, 2048], BF16, name=f"kts{i}") for i in range(4)]
        kt_l = [big.tile([128, 2048], BF16, name=f"ktl{i}") for i in range(4)]
        for i in range(4):
            nc.sync.dma_start(kt_s[i][D:D + 1, :],
                              ones_ap[:, i * 2048:(i + 1) * 2048])
        rhl_sb = const.tile([128, D], BF16)
        nc.sync.dma_start(rhl_sb[:], rhl_ap[:])
        rh_sb = const.tile([D, D], BF16)
        nc.sync.dma_start(rh_sb[:], rh_ap[:])
        ehl_sb = const.tile([128, D], BF16)
        nc.sync.dma_start(ehl_sb[:], ehl_ap[:])
        eh_sb = const.tile([D, D], BF16)
        nc.sync.dma_start(eh_sb[:], eh_ap[:])
        xqhh_sb = big.tile([128, NQ], BF16)
        nc.sync.dma_start(xqhh_sb[:], xqhh_ap[:])
        xqlo_sb = big.tile([D, NQ], BF16)
        nc.sync.dma_start(xqlo_sb[:], xqlo_ap[:])
        # key-side x, 8 chunk-tiles of 1024 so K projections start early
        xthh_sb = [big.tile([128, 1024], BF16, name=f"xthh{i}") for i in range(8)]
        xtlo_sb = [big.tile([D, 1024], BF16, name=f"xtlo{i}") for i in range(8)]
        for i in range(8):
            sl = slice(i * 1024, (i + 1) * 1024)
            nc.sync.dma_start(xthh_sb[i][:], xthh_ap[:, sl])
            nc.sync.dma_start(xtlo_sb[i][:], xtlo_ap[:, sl])
        ident = const.tile([128, 128], F32)
        nc.sync.dma_start(ident[:], id_ap[:])
        # xaug pre-packed on host into SBUF layout [128, block*72]
        xaug_sb = big.tile([128, NKB * DP], BF16)
        nc.sync.dma_start(xaug_sb[:], xaug_ap[:])
        xaug_v = xaug_sb[:].rearrange("p (t d) -> p t d", d=DP)[:, :, 0:D + 1]

        qst_s = big.tile([D + 1, NQ], BF16)     # Q_hi rows 0..63, -m row 64
        qst2 = big.tile([128, NQ], BF16)        # Q_lo rows 0..63, Q_hi 64..127

        # ---------------- projections ----------------
        for s in range(NQ // 512):
            pq_t = ppool.tile([128, 1024], F32, tag="pp", name="pq")
            pq = pq_t[0:D, 0:512]
            sl = slice(s * 512, (s + 1) * 512)
            nc.tensor.matmul(pq, rhl_sb[:], xqhh_sb[:, sl], start=True, stop=False)
            nc.tensor.matmul(pq, rh_sb[:], xqlo_sb[:, sl], start=False, stop=True)
            nc.scalar.copy(qst_s[0:D, sl], pq)
            nc.vector.tensor_copy(qst2[D:128, sl], qst_s[0:D, sl])
            nc.vector.tensor_tensor(out=qst2[0:D, sl], in0=pq,
                                    in1=qst_s[0:D, sl], op=SUB)
        for s in range(N // 512):
            qtr, off = divmod(s, 4)
            pk_t = ppool.tile([128, 1024], F32, tag="pp", name="pk")
            pk = pk_t[0:D, 0:512]
            xi, xo = divmod(s, 2)
            xsl = slice(xo * 512, (xo + 1) * 512)
            sl = slice(off * 512, (off + 1) * 512)
            nc.tensor.matmul(pk, ehl_sb[:], xthh_sb[xi][:, xsl],
                             start=True, stop=False)
            nc.tensor.matmul(pk, eh_sb[:], xtlo_sb[xi][:, xsl],
                             start=False, stop=True)
            nc.scalar.copy(kt_s[qtr][0:D, sl], pk)
            nc.vector.tensor_copy(kt_l[qtr][0:D, sl], kt_s[qtr][0:D, sl])
            nc.vector.tensor_tensor(out=kt_l[qtr][D:128, sl], in0=pk,
                                    in1=kt_s[qtr][0:D, sl], op=SUB)

        # ---------------- pass 1 ----------------
        mx = {}          # per chunk: [128, 36] f32, col rt = -m for row-tile rt
        mxp = {}         # per (chunk, rt): [128, 8] group maxes / lse sums

        def emit_p1_group(qc, rt, g, eng):
            q0 = qc * QC + rt * 128
            if g == 0:
                mxp[(qc, rt)] = work.tile([128, 8], F32, tag="mxp", name="mxp")
            ps1_t = ppool.tile([128, 1024], F32, tag="pp", name="ps1")
            for h in range(2):
                k0 = g * 1024 + h * 512
                qtr, ko = k0 // 2048, k0 % 2048
                nc.tensor.matmul(ps1_t[:, h * 512:(h + 1) * 512],
                                 qst_s[0:D, q0:q0 + 128],
                                 kt_s[qtr][0:D, ko:ko + 512],
                                 start=True, stop=True)
            if eng == "dve":
                nc.vector.reduce_max(mxp[(qc, rt)][:, g:g + 1], ps1_t[:], axis=AX)
            else:
                scr = work.tile([128, 1024], BF16, tag="lsescr", name="lsescr")
                nc.scalar.activation(scr[:], ps1_t[:], EXP, scale=1.0 / T_LSE,
                                     accum_out=mxp[(qc, rt)][:, g:g + 1])

        def finish_p1_rt(qc, rt, eng):
            if qc not in mx:
                mx[qc] = work.tile([128, 36], F32, tag="mx", name="mx")
                nc.vector.memset(mx[qc][:], 0.0)
            if eng == "dve":
                nc.vector.reduce_max(mx[qc][:, rt:rt + 1], mxp[(qc, rt)][:],
                                     axis=AX, negate=True)
            else:
                ssum = small.tile([128, 1], F32, tag="ssum", name="ssum")
                nc.vector.reduce_sum(ssum[:], mxp[(qc, rt)][:], axis=AX)
                lnv = small.tile([128, 1], F32, tag="lnv", name="lnv")
                # ACT's Ln spline is wrong for huge inputs (breaks ~1e20);
                # fold a 2^-64 pre-scale in and add 64*ln2 back after.
                nc.scalar.activation(lnv[:], ssum[:], LN, scale=2.0 ** -64)
                nc.scalar.activation(mx[qc][:, rt:rt + 1], lnv[:],
                                     mybir.ActivationFunctionType.Copy,
                                     bias=-T_LSE * 64.0 * float(np.log(2.0)),
                                     scale=-T_LSE)

        def emit_wb(qc, rt):
            pm_t = ppool.tile([128, 1024], F32, tag="pp", name="pm")
            ps_m = pm_t[0:32, 0:128]
            nc.tensor.transpose(ps_m, mx[qc][:, rt:rt + 32], ident[:])
            q0 = qc * QC + rt * 128
            nc.vector.tensor_copy(qst_s[D:D + 1, q0:q0 + 128], ps_m[0:1, :])

        # chunk-0 pass 1: row-tiles 0,1 on DVE and 2,3 on ACT, interleaved
        # pairwise so both engines run in parallel through the serial prefix
        for pair in ((0, 2), (1, 3)):
            for g in range(8):
                emit_p1_group(0, pair[0], g, "dve")
                emit_p1_group(0, pair[1], g, "lse")
            finish_p1_rt(0, pair[0], "dve")
            finish_p1_rt(0, pair[1], "lse")
        for rt in range(4):
            emit_wb(0, rt)

        # ---------------- main loop ----------------
        # 64 units of 2 key-blocks; chunk 1's pass-1 (32 all-DVE groups) is
        # interleaved into chunk 0's units 0..27 (units 0..3 take 2 groups),
        # its writeback lands at units 28..31, always before unit 32's score
        # matmuls get emitted.
        p1c1 = [(rt, g) for rt in range(4) for g in range(8)]
        sched = {}
        pos = 0
        for u in range(28):
            take = 2 if u < 4 else 1
            sched[u] = p1c1[pos:pos + take]
            pos += take

        def emit_score(qc, jj):
            pexp_t = ppool.tile([128, 1024], F32, tag="pp", name="pexp")
            qsl = slice(qc * QC, (qc + 1) * QC)
            for h in range(2):
                j = jj + h
                qtr, jo = divmod(j, 16)
                blk = slice(jo * 128, (jo + 1) * 128)
                reg = pexp_t[:, h * 512:(h + 1) * 512]
                nc.tensor.matmul(reg, kt_s[qtr][:, blk], qst_s[:, qsl],
                                 start=True, stop=False)
                nc.tensor.matmul(reg, kt_l[qtr][:, blk], qst2[:, qsl],
                                 start=False, stop=True)
            return pexp_t

        po = {}
        pexp_cur = emit_score(0, 0)
        for u in range(2 * NKB // 2):
            qc, jj = u // 32, 2 * (u % 32)
            if jj == 0:
                po[qc] = pacc.tile([D + 1, QC], F32, tag="po", name="po")
            pt = work.tile([128, 1024], BF16, tag="pt", name="pt")
            nc.scalar.activation(pt[:], pexp_cur[:], EXP)
            if u + 1 < 64:
                pexp_cur = emit_score((u + 1) // 32, 2 * ((u + 1) % 32))
            nc.tensor.matmul(po[qc][:], xaug_v[:, jj, :], pt[:, 0:512],
                             start=(jj == 0), stop=False)
            nc.tensor.matmul(po[qc][:], xaug_v[:, jj + 1, :], pt[:, 512:1024],
                             start=False, stop=(jj == 62))
            if qc == 0:
                for (rt, g) in sched.get(u, ()):
                    emit_p1_group(1, rt, g, "dve")
                    if g == 7:
                        finish_p1_rt(1, rt, "dve")
                if 28 <= u <= 31:
                    emit_wb(1, u - 28)
            if jj == 62:
                # normalize chunk qc: out[q,:] = po[0:64,q] / po[64,q]
                ot = work.tile([D + 1, QC], F32, tag="ot", name="ot")
                nc.vector.tensor_copy(ot[:], po[qc][:])
                for h in range(QC // 128):
                    ptr_t = ppool.tile([128, 1024], F32, tag="pp", name="ptr")
                    ps_t = ptr_t[0:128, 0:D + 1]
                    nc.tensor.transpose(ps_t, ot[:, h * 128:(h + 1) * 128],
                                        ident[0:D + 1, 0:D + 1])
                    recip = small.tile([128, 1], F32, tag="recip", name="recip")
                    nc.vector.reciprocal(recip[:], ps_t[:, D:D + 1])
                    o_sb = small.tile([128, D], F32, tag="osb", name="osb")
                    nc.vector.tensor_scalar_mul(o_sb[:], ps_t[:, 0:D], recip[:])
                    r0 = qc * QC + h * 128
                    nc.sync.dma_start(out_ap[r0:r0 + 128, :], o_sb[:])

        nc.sync.dma_start(dbg_ap[:], qst_s[D:D + 1, :])

    nc.compile()
    return nc


_CACHE = {}


def _get_nc():
    if "nc" not in _CACHE:
        _CACHE["nc"] = build()
    return _CACHE["nc"]


def kernel(x, rotation_params, entangle_params, _trace=False, _nc=None):
    from concourse.bass_utils import run_bass_kernel_spmd
    import ml_dtypes

    bf16 = ml_dtypes.bfloat16
    f32 = np.float32

    x = np.ascontiguousarray(x, dtype=f32)
    rs = np.ascontiguousarray(rotation_params, dtype=f32) / 8.0
    e = np.ascontiguousarray(entangle_params, dtype=f32)

    xh = x.astype(bf16)
    xl = (x - xh.astype(f32)).astype(bf16)
    xthh = np.ascontiguousarray(np.vstack([xh.T, xh.T]))          # [128, N]
    xtlo = np.ascontiguousarray(xl.T)                             # [64, N]

    def hl(w):
        h = w.astype(bf16)
        l = (w - h.astype(f32)).astype(bf16)
        return np.ascontiguousarray(np.vstack([h, l])), h

    rhl, rh = hl(rs)
    ehl, eh = hl(e)

    xaug = np.zeros((N, DP), dtype=bf16)
    xaug[:, :D] = xh
    xaug[:, D] = 1.0
    # host pre-pack into the SBUF layout [128, block*72]
    xaug_p = np.ascontiguousarray(
        xaug.reshape(NKB, 128, DP).transpose(1, 0, 2).reshape(128, NKB * DP))

    ones16 = np.ones((1, N), dtype=bf16)
    ident = np.eye(128, dtype=f32)

    nc = _nc if _nc is not None else _get_nc()
    in_maps = []
    for c in range(NCORES):
        qsl = slice(c * NQ, (c + 1) * NQ)
        in_maps.append({
            "ones16": ones16,
            "rhl": rhl, "rh": rh, "ehl": ehl, "eh": eh,
            "xqhh": np.ascontiguousarray(xthh[:, qsl]),
            "xqlo": np.ascontiguousarray(xtlo[:, qsl]),
            "xthh": xthh, "xtlo": xtlo,
            "ident": ident,
            "xaug": xaug_p,
        })
    res = run_bass_kernel_spmd(nc, in_maps, core_ids=list(range(NCORES)),
                               trace=_trace)
    out = np.concatenate([res.results[c]["out"] for c in range(NCORES)], axis=0)
    if _trace:
        return out, res
    return out


# revision 6
# speedup vs baseline: 1.5623x; 1.3123x over previous
"""Trainium2 Bass kernel for the AttentionBlock problem (v3).

Full inputs -> full output; sharded over 8 NeuronCores (core c owns
queries [1024*c, 1024*(c+1))); every core gets the full key-side x, so
no on-device collectives.

Two hardware facts drive the structure (both measured from NTFF traces
on this part):
  1. All matmul dtypes stream 1 column/cycle; the PE clock is 1.2 GHz
     while the Vector engine is active and 2.4 GHz when DVE is quiet
     (shared power cap: DVE activity clamps the PE clock).  So ALL
     DVE work (projection lo-splits, max-pass reduces, normalize) is
     packed into a prefix/tail, and the main score*V loop runs with the
     vector engine silent -> 2.4 GHz matmuls, ~2x faster.
  2. fp32 matmuls run ~2.5x slower (LOW_HIGH) -> everything on the PE
     is bf16, with hi/lo splits for accuracy.

Math (per core, N=8192 keys, Nq=1024 queries, d=64):
  projections (bf16 hi/lo, x pre-split on host):
    Q = x@R/8 = [Rh;Rl]^T [xh;xh] + Rh^T xl     (2 MMs / 512 cols)
  pass-1 (max estimate), 2x row-group packed: row groups 0-1 compute
    K_hi.Q_hi for keys [k, k+512), groups 2-3 for [k+512, k+1024)
    concurrently (contraction is only 64) -> half the PE passes.
    Per-query m: even row-tiles DVE reduce_max; odd row-tiles ACT
    log-sum-exp (T=10, exp accum_out; m = T*ln(sum) in [max, max+28])
    with ln computed from the fp32 exponent bits (no Ln table load).
  scores, 2 bf16 MMs per 128-key block into PSUM:
    MM1 = [K_hi; 1]^T [Q_hi; -m]        (65 rows, carries the shift)
    MM2 = [K_lo; K_hi]^T [Q_hi; Q_lo]   (full lo-cross terms)
  P = bf16(exp(S)) - bf16's e^88 range absorbs the lse overshoot
  PV: out_aug^T += xaug_j^T P_j   (xaug = [x_bf16, 1])
  normalize (deferred to tail): out = out_aug[0:64] / out_aug[64]

Numerics of this exact quantization, CPU-simulated: rel err 3.1e-3
(gate 2e-2).
"""

import numpy as np
from contextlib import ExitStack

import concourse.bass as bass
import concourse.tile as tile
from concourse import bacc, mybir

N = 8192
D = 64
NCORES = 8
NQ = N // NCORES          # 1024 queries per core
NKB = N // 128            # 64 key blocks
QC = 512                  # query chunk (pass-2 free dim)
DP = 72                   # xaug row stride (16B aligned in bf16)
T_LSE = 10.0              # lse temperature for the ACT max-proxy
LN2 = 0.6931471805599453

F32 = mybir.dt.float32
BF16 = mybir.dt.bfloat16
I32 = mybir.dt.int32
AX = mybir.AxisListType.X
SUB = mybir.AluOpType.subtract
EXP = mybir.ActivationFunctionType.Exp
COPY = mybir.ActivationFunctionType.Copy


def build():
    nc = bacc.Bacc("TRN2", target_bir_lowering=False, debug=False, num_devices=1)

    ones_ap = nc.dram_tensor("ones16", [1, N], BF16, kind="ExternalInput").ap()
    rhl_ap = nc.dram_tensor("rhl", [128, D], BF16, kind="ExternalInput").ap()
    rh_ap = nc.dram_tensor("rh", [D, D], BF16, kind="ExternalInput").ap()
    ehl_ap = nc.dram_tensor("ehl", [128, D], BF16, kind="ExternalInput").ap()
    eh_ap = nc.dram_tensor("eh", [D, D], BF16, kind="ExternalInput").ap()
    xqhh_ap = nc.dram_tensor("xqhh", [128, NQ], BF16, kind="ExternalInput").ap()
    xqlo_ap = nc.dram_tensor("xqlo", [D, NQ], BF16, kind="ExternalInput").ap()
    xthh_ap = nc.dram_tensor("xthh", [128, N], BF16, kind="ExternalInput").ap()
    xtlo_ap = nc.dram_tensor("xtlo", [D, N], BF16, kind="ExternalInput").ap()
    id_ap = nc.dram_tensor("ident", [128, 128], F32, kind="ExternalInput").ap()
    xaug_ap = nc.dram_tensor("xaug", [128, NKB * DP], BF16,
                             kind="ExternalInput").ap()
    out_ap = nc.dram_tensor("out", [NQ, D], F32, kind="ExternalOutput").ap()

    with tile.TileContext(nc) as tc, ExitStack() as ctx:
        const = ctx.enter_context(tc.tile_pool(name="const", bufs=1))
        big = ctx.enter_context(tc.tile_pool(name="big", bufs=1))
        work = ctx.enter_context(tc.tile_pool(name="work", bufs=3))
        small = ctx.enter_context(tc.tile_pool(name="small", bufs=2))
        # PSUM: ppool 3x[128,1024] = 6 banks, pacc 2x[65,512] = 2 banks
        ppool = ctx.enter_context(tc.tile_pool(name="pp", bufs=3, space="PSUM"))
        pacc = ctx.enter_context(tc.tile_pool(name="pacc", bufs=2, space="PSUM"))

        # ---------------- tiles + input DMA ----------------
        # kt_s quarters [65, 2048]: K_hi rows 0..63, ones row 64.
        # kt_l quarters [128, 2048]: K_lo rows 0..63, K_hi rows 64..127
        # (hi on the upper rows doubles as the rhs of the packed pass-1).
        kt_s = [big.tile([D + 1, 2048], BF16, name=f"kts{i}") for i in range(4)]
        kt_l = [big.tile([128, 2048], BF16, name=f"ktl{i}") for i in range(4)]
        for i in range(4):
            nc.sync.dma_start(kt_s[i][D:D + 1, :],
                              ones_ap[:, i * 2048:(i + 1) * 2048])
        rhl_sb = const.tile([128, D], BF16)
        nc.sync.dma_start(rhl_sb[:], rhl_ap[:])
        rh_sb = const.tile([D, D], BF16)
        nc.sync.dma_start(rh_sb[:], rh_ap[:])
        ehl_sb = const.tile([128, D], BF16)
        nc.sync.dma_start(ehl_sb[:], ehl_ap[:])
        eh_sb = const.tile([D, D], BF16)
        nc.sync.dma_start(eh_sb[:], eh_ap[:])
        xqhh_sb = big.tile([128, NQ], BF16)
        nc.sync.dma_start(xqhh_sb[:], xqhh_ap[:])
        xqlo_sb = big.tile([D, NQ], BF16)
        nc.sync.dma_start(xqlo_sb[:], xqlo_ap[:])
        # key-side x, 8 chunk-tiles of 1024 so K projections start early
        xthh_sb = [big.tile([128, 1024], BF16, name=f"xthh{i}") for i in range(8)]
        xtlo_sb = [big.tile([D, 1024], BF16, name=f"xtlo{i}") for i in range(8)]
        for i in range(8):
            sl = slice(i * 1024, (i + 1) * 1024)
            nc.sync.dma_start(xthh_sb[i][:], xthh_ap[:, sl])
            nc.sync.dma_start(xtlo_sb[i][:], xtlo_ap[:, sl])
        ident = const.tile([128, 128], F32)
        nc.sync.dma_start(ident[:], id_ap[:])
        # xaug pre-packed on host into SBUF layout [128, block*72]
        xaug_sb = big.tile([128, NKB * DP], BF16)
        nc.sync.dma_start(xaug_sb[:], xaug_ap[:])
        xaug_v = xaug_sb[:].rearrange("p (t d) -> p t d", d=DP)[:, :, 0:D + 1]

        qst_s = big.tile([D + 1, NQ], BF16)   # Q_hi rows 0..63, -m row 64
        qst2 = big.tile([128, NQ], BF16)      # Q_hi rows 0..63, Q_lo 64..127
        qhh2 = big.tile([128, NQ], BF16)      # Q_hi rows 64..127 (pass-1 lhsT)

        # ---------------- projections ----------------
        for s in range(NQ // 512):
            pq_t = ppool.tile([128, 1024], F32, tag="pp", name="pq")
            pq = pq_t[0:D, 0:512]
            sl = slice(s * 512, (s + 1) * 512)
            nc.tensor.matmul(pq, rhl_sb[:], xqhh_sb[:, sl], start=True, stop=False)
            nc.tensor.matmul(pq, rh_sb[:], xqlo_sb[:, sl], start=False, stop=True)
            nc.scalar.copy(qst_s[0:D, sl], pq)
            nc.vector.tensor_copy(qst2[0:D, sl], qst_s[0:D, sl])
            nc.vector.tensor_copy(qhh2[D:128, sl], qst_s[0:D, sl])
            nc.vector.tensor_tensor(out=qst2[D:128, sl], in0=pq,
                                    in1=qst_s[0:D, sl], op=SUB)
        for s in range(N // 512):
            qtr, off = divmod(s, 4)
            pk_t = ppool.tile([128, 1024], F32, tag="pp", name="pk")
            pk = pk_t[0:D, 0:512]
            xi, xo = divmod(s, 2)
            xsl = slice(xo * 512, (xo + 1) * 512)
            sl = slice(off * 512, (off + 1) * 512)
            nc.tensor.matmul(pk, ehl_sb[:], xthh_sb[xi][:, xsl],
                             start=True, stop=False)
            nc.tensor.matmul(pk, eh_sb[:], xtlo_sb[xi][:, xsl],
                             start=False, stop=True)
            nc.scalar.copy(kt_s[qtr][0:D, sl], pk)
            nc.vector.tensor_copy(kt_l[qtr][D:128, sl], kt_s[qtr][0:D, sl])
            nc.vector.tensor_tensor(out=kt_l[qtr][0:D, sl], in0=pk,
                                    in1=kt_s[qtr][0:D, sl], op=SUB)

        # ---------------- pass 1 (both chunks, in the prefix) ----------
        # 8 row-tiles of 128 queries; per (rt, c) slot two row-group-packed
        # MMs cover keys [1024c, 1024c+512) and [1024c+512, 1024(c+1)).
        mx = work.tile([128, 40], F32, tag="mx", name="mx")
        nc.vector.memset(mx[:], 0.0)
        mxp = {}

        def emit_p1_slot(rt, c, eng):
            q0 = rt * 128
            if c == 0:
                mxp[rt] = work.tile([128, 8], F32, tag="mxp", name="mxp")
            ps1_t = ppool.tile([128, 1024], F32, tag="pp", name="ps1")
            k0 = c * 1024
            qtr_a, ko_a = divmod(k0, 2048)
            nc.tensor.matmul(ps1_t[:, 0:512],
                             qst_s[0:D, q0:q0 + 128],
                             kt_s[qtr_a][0:D, ko_a:ko_a + 512],
                             start=True, stop=True)
            qtr_b, ko_b = divmod(k0 + 512, 2048)
            nc.tensor.matmul(ps1_t[:, 512:1024],
                             qhh2[D:128, q0:q0 + 128],
                             kt_l[qtr_b][D:128, ko_b:ko_b + 512],
                             start=True, stop=True, tile_position=(64, 0))
            if eng == "dve":
                nc.vector.reduce_max(mxp[rt][:, c:c + 1], ps1_t[:], axis=AX)
            else:
                scr = work.tile([128, 1024], BF16, tag="lsescr", name="lsescr")
                nc.scalar.activation(scr[:], ps1_t[:], EXP, scale=1.0 / T_LSE,
                                     accum_out=mxp[rt][:, c:c + 1])

        def finish_p1_rt(rt, eng):
            if eng == "dve":
                nc.vector.reduce_max(mx[:, rt:rt + 1], mxp[rt][:],
                                     axis=AX, negate=True)
            else:
                # -m = -T*ln(ssum); ln from the fp32 exponent bits
                # (ACT's Ln table is wrong for huge inputs and would
                # thrash the Exp table set anyway):
                # bits(s)*2^-23 ~= log2(s) + 127  (within +0.086)
                ssum = small.tile([128, 1], F32, tag="ssum", name="ssum")
                nc.vector.reduce_sum(ssum[:], mxp[rt][:], axis=AX)
                ibits = small.tile([128, 1], F32, tag="ibits", name="ibits")
                nc.vector.tensor_copy(ibits[:], ssum[:].bitcast(I32))
                nc.scalar.activation(mx[:, rt:rt + 1], ibits[:], COPY,
                                     scale=-T_LSE * LN2 * 2.0 ** -23,
                                     bias=127.0 * T_LSE * LN2)

        for pr in range(4):
            for c in range(8):
                emit_p1_slot(2 * pr, c, "dve")
                emit_p1_slot(2 * pr + 1, c, "lse")
            finish_p1_rt(2 * pr, "dve")
            finish_p1_rt(2 * pr + 1, "lse")

        for rt in range(8):
            pm_t = ppool.tile([128, 1024], F32, tag="pp", name="pm")
            ps_m = pm_t[0:32, 0:128]
            nc.tensor.transpose(ps_m, mx[:, rt:rt + 32], ident[:])
            q0 = rt * 128
            nc.vector.tensor_copy(qst_s[D:D + 1, q0:q0 + 128], ps_m[0:1, :])

        # ---------------- main loop (vector engine silent) -------------
        def emit_score(qc, jj):
            pexp_t = ppool.tile([128, 1024], F32, tag="pp", name="pexp")
            qsl = slice(qc * QC, (qc + 1) * QC)
            for h in range(2):
                j = jj + h
                qtr, jo = divmod(j, 16)
                blk = slice(jo * 128, (jo + 1) * 128)
                reg = pexp_t[:, h * 512:(h + 1) * 512]
                nc.tensor.matmul(reg, kt_s[qtr][:, blk], qst_s[:, qsl],
                                 start=True, stop=False)
                nc.tensor.matmul(reg, kt_l[qtr][:, blk], qst2[:, qsl],
                                 start=False, stop=True)
            return pexp_t

        po = {}
        pexp_cur = emit_score(0, 0)
        for u in range(64):
            qc, jj = u // 32, 2 * (u % 32)
            if jj == 0:
                po[qc] = pacc.tile([D + 1, QC], F32, tag="po", name="po")
            pt = work.tile([128, 1024], BF16, tag="pt", name="pt")
            nc.scalar.activation(pt[:], pexp_cur[:], EXP)
            if u + 1 < 64:
                pexp_cur = emit_score((u + 1) // 32, 2 * ((u + 1) % 32))
            nc.tensor.matmul(po[qc][:], xaug_v[:, jj, :], pt[:, 0:512],
                             start=(jj == 0), stop=False)
            nc.tensor.matmul(po[qc][:], xaug_v[:, jj + 1, :], pt[:, 512:1024],
                             start=False, stop=(jj == 62))

        # ---------------- normalize (tail) ----------------
        for qc in range(2):
            ot = work.tile([D + 1, QC], F32, tag="ot", name="ot")
            nc.vector.tensor_copy(ot[:], po[qc][:])
            for h in range(QC // 128):
                ptr_t = ppool.tile([128, 1024], F32, tag="pp", name="ptr")
                ps_t = ptr_t[0:128, 0:D + 1]
                nc.tensor.transpose(ps_t, ot[:, h * 128:(h + 1) * 128],
                                    ident[0:D + 1, 0:D + 1])
                recip = small.tile([128, 1], F32, tag="recip", name="recip")
                nc.vector.reciprocal(recip[:], ps_t[:, D:D + 1])
                o_sb = small.tile([128, D], F32, tag="osb", name="osb")
                nc.vector.tensor_scalar_mul(o_sb[:], ps_t[:, 0:D], recip[:])
                r0 = qc * QC + h * 128
                nc.sync.dma_start(out_ap[r0:r0 + 128, :], o_sb[:])

    nc.compile()
    return nc


_CACHE = {}


def _get_nc():
    if "nc" not in _CACHE:
        _CACHE["nc"] = build()
    return _CACHE["nc"]


def kernel(x, rotation_params, entangle_params, _trace=False, _nc=None):
    from concourse.bass_utils import run_bass_kernel_spmd
    import ml_dtypes

    bf16 = ml_dtypes.bfloat16
    f32 = np.float32

    x = np.ascontiguousarray(x, dtype=f32)
    rs = np.ascontiguousarray(rotation_params, dtype=f32) / 8.0
    e = np.ascontiguousarray(entangle_params, dtype=f32)

    xh = x.astype(bf16)
    xl = (x - xh.astype(f32)).astype(bf16)
    xthh = np.ascontiguousarray(np.vstack([xh.T, xh.T]))          # [128, N]
    xtlo = np.ascontiguousarray(xl.T)                             # [64, N]

    def hl(w):
        h = w.astype(bf16)
        l = (w - h.astype(f32)).astype(bf16)
        return np.ascontiguousarray(np.vstack([h, l])), h

    rhl, rh = hl(rs)
    ehl, eh = hl(e)

    xaug = np.zeros((N, DP), dtype=bf16)
    xaug[:, :D] = xh
    xaug[:, D] = 1.0
    xaug_p = np.ascontiguousarray(
        xaug.reshape(NKB, 128, DP).transpose(1, 0, 2).reshape(128, NKB * DP))

    ones16 = np.ones((1, N), dtype=bf16)
    ident = np.eye(128, dtype=f32)

    nc = _nc if _nc is not None else _get_nc()
    in_maps = []
    for c in range(NCORES):
        qsl = slice(c * NQ, (c + 1) * NQ)
        in_maps.append({
            "ones16": ones16,
            "rhl": rhl, "rh": rh, "ehl": ehl, "eh": eh,
            "xqhh": np.ascontiguousarray(xthh[:, qsl]),
            "xqlo": np.ascontiguousarray(xtlo[:, qsl]),
            "xthh": xthh, "xtlo": xtlo,
            "ident": ident,
            "xaug": xaug_p,
        })
    res = run_bass_kernel_spmd(nc, in_maps, core_ids=list(range(NCORES)),
                               trace=_trace)
    out = np.concatenate([res.results[c]["out"] for c in range(NCORES)], axis=0)
    if _trace:
        return out, res
    return out
